# revision 31
# baseline (speedup 1.0000x reference)
"""Trainium2 Bass kernel for nn_ClassificationLayer (Gaussian pdf-sum classifier).

Math:
  mu/sd per dim from tiny [128,10] reference sets (host, exact).
  Per row i: s_n[i] = sum_d INV_SQRT_2PI/sd_d * exp(-0.5*((x[i,d]-mu_d)/sd_d)^2)
  (same for anomaly), then the batch recurrence p_k = (p_{k-1} + s_k)/128,
  output = [pn/(pn+pa), pa/(pn+pa)].

Device strategy (8 cores, data-parallel over N, exact 62500-row shards):
  - Host transposes each core's shard to [128 dims, R rows]; per-dim constants
    become per-partition scale/bias vectors.
  - The elementwise Gaussian is split across TWO engines so neither is the
    bottleneck:
      * ScalarE: one ACTIVATE per distribution per tile computes
        Derivative_Erf(scale*x + bias) = (2/sqrt(pi)) * exp(-((x-mu)/sd)^2/2),
        output in bf16.  ~1.04 ns per column per distribution.
      * VectorE: a custom fused DVE op (registered at import) computes
        Schraudolph exp bits: out_i16 = max(BETA - (a'x + b')^2, 0) converted
        to int16, which *is* the bf16 bit pattern of ~exp(-((x-mu)/sd)^2/2).
        ~1.26 ns per column per distribution.  Max per-element error ~3%
        (sawtooth); uniform bias cancels in the output ratio.
  - Reduction over dims (partitions) via TensorEngine matvec in bf16.  The
    stationary operand is a 32-wide shifted window over a zero-padded bf16
    weight buffer so chunk g's sums land in PSUM row g%32 of its 32-chunk
    accumulation group (matmul output base partitions are restricted to
    {0,32,64}, so groups are 32 wide).  Groups [0,32), [32,64), [64,96)
    drain under compute on the SWDGE queue as each completes; only a
    [27,512] group per dist remains at the end.
  - The x stream owns the Sync HWDGE queue exclusively (descriptors in pure
    stream order); consts ride the Scalar HWDGE queue; weights are [128,4]
    bf16 (512B).  Final two 14KB drains go on sync+scalar HWDGE in parallel.
  - The scalar recurrence decays by 1/128 per step, so it is re-run exactly on
    the gathered per-row sums on host as a short causal convolution in float64.
"""

import numpy as np

N, DIM, S = 500000, 128, 10
INV_SQRT_2PI = 0.3989422804014327
NCORES = 8
CHUNK = 512                      # rows per matvec (PSUM bank free-dim)
R = N // NCORES                  # 62500 rows per core, exact (no padding)
NCHUNK = -(-R // CHUNK)          # 123 chunks; last chunk has 36 rows
LAST_W = R - (NCHUNK - 1) * CHUNK

# Tile plan: (n_chunks, n_dve_chunks). ScalarE takes the FIRST nc-nd chunks
# of each tile, DVE the rest; each half is DMA'd separately so each engine
# only waits for its own half (halves the pipeline granularity). Engine
# per-column cost degrades beyond ~2560 cols/instruction, so 8-chunk tiles
# (2048-col instructions) are the sweet spot; small head tiles start both
# engines early, small tail tiles keep the post-stream compute lag tiny.
# DVE share 56/123 balances DVE (~1.09 marginal ns/col) vs ScalarE (~0.88).
TILE_PLAN = ([(2, 1), (4, 2)]
             + [(8, 4), (8, 3), (8, 4), (8, 4), (8, 3), (8, 4), (8, 4),
                (8, 3), (8, 4), (8, 4), (8, 3), (8, 4), (8, 3)]
             + [(6, 3), (4, 2), (2, 1), (1, 0)])
assert sum(t[0] for t in TILE_PLAN) == NCHUNK
assert sum(t[1] for t in TILE_PLAN) == 56
TILES = []           # (chunk0, n_chunks, n_dve)
_g = 0
for _nc_, _nd in TILE_PLAN:
    TILES.append((_g, _nc_, _nd))
    _g += _nc_
MAX_WV = max(nd * CHUNK for _, nd in TILE_PLAN)
MAX_WS = max((nc_ - nd) * CHUNK for nc_, nd in TILE_PLAN)
MAX_W = max(nc_ * CHUNK for nc_, _ in TILE_PLAN)

# PSUM accumulation groups: 32 chunks each (matmul base partitions must be
# in {0,32,64}); chunk g lands in row g%32 of group g//32. Groups 0-2 drain
# mid-stream on the SWDGE queue; group 3 ([27,512]) is the small final drain.
GROUPS = [(0, 32), (32, 64), (64, 96), (96, NCHUNK)]     # [base, end)
NGRP = len(GROUPS)
# tile index after which each group's chunks are all emitted
_cum = 0
GROUP_FLUSH_TILE = {}
for _ti, (_, _nc_, _) in enumerate(TILES):
    _cum += _nc_
    for _gi, (_b, _e) in enumerate(GROUPS):
        if _cum >= _e and _gi not in GROUP_FLUSH_TILE:
            GROUP_FLUSH_TILE[_gi] = _ti

# weight-window bases in the [128, 256] weight buffer (c vector at the base
# column, zeros elsewhere; window [base-r, base-r+32) puts c at PSUM row r;
# adjacent windows span [base-31, base+32) and never reach another base)
ACT_N_BASE, ACT_A_BASE, DVE_N_BASE, DVE_A_BASE = 32, 96, 160, 224
W_COLS = 256

# Schraudolph-in-bf16-bit-space constants.
LOG2E = 1.4426950408889634
K_DVE = float(np.sqrt(128.0 * LOG2E))       # folds 2^7*log2(e) into a', b'
C_CENTER = 1.5 - 1.0 / float(np.log(2.0))   # zero-mean sawtooth offset
BETA = float(128.0 * (127.0 - C_CENTER))

_COMPILED = None
_DVE_OP = None
LAST_RESULTS = None  # BassKernelResults of the most recent device run


def _register_dve_op():
    """Register the fused Schraudolph-exp custom DVE op at runtime."""
    global _DVE_OP
    if _DVE_OP is not None:
        return _DVE_OP
    import concourse.dve_ops as dvo
    from concourse.dve_spec import Spec, Src0, C0, C1, C2, Zero, maxx, sq, lower
    from concourse.dve_spec import _has_src1
    from concourse.dve_uop import DveOpSpec

    name = "GAUSS_EXP_BITS_ANT"
    if name in dvo._SUB_OPCODE_FOR_NAME:
        _DVE_OP = next(op for op in dvo.OPS if op.name == name)
        return _DVE_OP
    t = Src0 * C0 + C1
    spec = Spec(
        body=maxx(C2 - sq(t), Zero),
        reference=lambda in0, in1, s0, s1, imm2: np.maximum(
            np.float32(imm2)
            - (in0 * s0 + s1).astype(np.float32) ** 2,
            np.float32(0.0),
        ),
    )
    row = dvo._CUSTOM_DVE_ROW_BASE + len(dvo.OPS)
    shas = {}
    for ver in ("v3", "v4"):
        try:
            uops = lower(spec, ver=ver)
            shas[ver] = DveOpSpec(
                name=name, opcode=row, uops=uops, rd1_en=_has_src1(spec)
            ).sha(ver)
        except Exception:
            pass
    op = dvo.DveOp(name, spec, subdim=False, uops_sha=shas)
    dvo.OPS.append(op)
    dvo._SUB_OPCODE_FOR_NAME[name] = row
    dvo.CUSTOM_DVE_SPECS[name] = spec
    _DVE_OP = op
    return op


def _build():
    import concourse.tile as tile
    from concourse import bacc, mybir

    dve_op = _register_dve_op()

    nc = bacc.Bacc("TRN2", target_bir_lowering=False, debug=False,
                   num_devices=NCORES)

    xT = nc.dram_tensor("xT", [DIM, R], mybir.dt.float32,
                        kind="ExternalInput").ap()
    # consts cols: 0 scale_n, 1 bias_n, 2 scale_a, 3 bias_a (ACT);
    #              4 a'_n, 5 b'_n, 6 a'_a, 7 b'_a (DVE, scaled by K_DVE)
    consts = nc.dram_tensor("consts", [DIM, 8], mybir.dt.float32,
                            kind="ExternalInput").ap()
    wmat = nc.dram_tensor("wmat", [DIM, W_COLS], mybir.dt.bfloat16,
                          kind="ExternalInput").ap()
    sn_out = nc.dram_tensor("sn_out", [128, CHUNK], mybir.dt.float32,
                            kind="ExternalOutput").ap()
    sa_out = nc.dram_tensor("sa_out", [128, CHUNK], mybir.dt.float32,
                            kind="ExternalOutput").ap()

    DErf = mybir.ActivationFunctionType.Derivative_Erf
    bf16 = mybir.dt.bfloat16

    with tile.TileContext(nc) as tc:
        with tc.tile_pool(name="cpool", bufs=1) as cpool, \
             tc.tile_pool(name="xpool", bufs=6) as xpool, \
             tc.tile_pool(name="evpool", bufs=3) as evpool, \
             tc.tile_pool(name="espool", bufs=3) as espool, \
             tc.tile_pool(name="pspool", bufs=1, space="PSUM") as pspool:

            # x tiles first on the Sync HWDGE queue: the stream's first
            # descriptor is x tile 0 (consts ride the Scalar HWDGE queue).
            # Each tile is fetched as two DMAs — ScalarE's half (cols
            # [0,ws)) then DVE's half — so ACT only waits for its own half,
            # which lands ~half a tile earlier in the stream.
            def fetch_x(ti, name=None):
                off, nch, ndv = TILES[ti]
                off *= CHUNK
                w = min(nch * CHUNK, R - off)
                ws = w - ndv * CHUNK
                x_t = xpool.tile([DIM, w], mybir.dt.float32, tag="x",
                                 padded_shape=[DIM, MAX_W],
                                 name=name or f"x_{ti}")
                if ws >= 2 * CHUNK and w - ws >= 2 * CHUNK:
                    nc.sync.dma_start(x_t[:, 0:ws], xT[:, off:off + ws])
                    nc.sync.dma_start(x_t[:, ws:w], xT[:, off + ws:off + w])
                else:
                    nc.sync.dma_start(x_t[:], xT[:, off:off + w])
                return x_t

            x_pre = {ti: fetch_x(ti, name=f"x_pre{ti}") for ti in (0, 1)}
            consts_t = cpool.tile([DIM, 8], mybir.dt.float32)
            nc.scalar.dma_start(consts_t[:], consts[:, :])
            # Dummy activation: triggers the erf_derivative table load while
            # the first x tiles are still in flight. memset instead of a
            # consts read so the table load has no DMA dependency at all.
            warm_t = cpool.tile([DIM, 1], mybir.dt.float32)
            nc.vector.memset(warm_t[:], 0.0)
            nc.scalar.activation(warm_t[:], warm_t[:], DErf,
                                 bias=0.0, scale=1.0)
            # weights via SWDGE, off both HWDGE queues
            w_t = cpool.tile([DIM, W_COLS], bf16)
            nc.gpsimd.dma_start(w_t[:], wmat[:, :])

            # per dist: two [64,512] PSUM tiles, each holding two 32-chunk
            # accumulation groups at base partitions 0 and 32 (both legal
            # matmul output bases); chunk g accumulates into row g%32 of
            # group g//32.
            ps = {}   # (dist, group) -> (tile, row_base)
            sb = {}   # (dist, group) -> staging tile
            for d in (0, 1):
                for half in (0, 1):
                    pt = pspool.tile([64, CHUNK], mybir.dt.float32,
                                     name=f"ps_{d}_{half}")
                    ps[(d, 2 * half)] = (pt, 0)
                    ps[(d, 2 * half + 1)] = (pt, 32)
                for gi in (2, 3):
                    b, e = GROUPS[gi]
                    sb[(d, gi)] = cpool.tile([e - b, CHUNK],
                                             mybir.dt.float32,
                                             name=f"sb_{d}_{gi}")
            sb01_n = cpool.tile([64, CHUNK], mybir.dt.float32)
            sb01_a = cpool.tile([64, CHUNK], mybir.dt.float32)

            # PE emission is ascending global chunk order per dist (ACT
            # chunks come first within each tile), so group gi's first/last
            # emitted chunks are simply its boundary chunks.
            flushed = set()
            for ti, (g0, nch, ndv) in enumerate(TILES):
                off = g0 * CHUNK
                w = min(nch * CHUNK, R - off)
                ns_ = nch - ndv
                ws = min(ns_ * CHUNK, w)
                wv = w - ws
                x_t = x_pre[ti] if ti in x_pre else fetch_x(ti)
                # Two waves per tile: produce n-dist results, run n matmuls
                # while the a-dist results are being produced, then a
                # matmuls. ACT chunks first within each wave. The last two
                # tiles process dist a FIRST so its slower final drain chain
                # (scalar copy -> SWDGE) starts earlier and both output
                # chains finish together.
                d_order = (1, 0) if ti >= len(TILES) - 2 else (0, 1)
                for dist in d_order:
                    ev = es = None
                    if ws:
                        es = espool.tile([DIM, ws], bf16,
                                         tag="es" + "na"[dist],
                                         padded_shape=[DIM, MAX_WS])
                        nc.scalar.activation(es[:], x_t[:, 0:ws], DErf,
                                             bias=consts_t[:, 1 + 2 * dist:
                                                           2 + 2 * dist],
                                             scale=consts_t[:, 2 * dist:
                                                            1 + 2 * dist])
                    if wv:
                        ev = evpool.tile([DIM, wv], bf16,
                                         tag="ev" + "na"[dist],
                                         padded_shape=[DIM, MAX_WV])
                        nc.vector._custom_dve(
                            dve_op, out=ev[:].bitcast(mybir.dt.int16),
                            in0=x_t[:, ws:w],
                            s0=consts_t[:, 4 + 2 * dist:5 + 2 * dist],
                            s1=consts_t[:, 5 + 2 * dist:6 + 2 * dist],
                            imm2=BETA)
                    base_v = DVE_N_BASE if dist == 0 else DVE_A_BASE
                    base_s = ACT_N_BASE if dist == 0 else ACT_A_BASE
                    for c in range(nch):
                        g = g0 + c
                        r = g % 32
                        cw = min(CHUNK, w - c * CHUNK)
                        if c < ns_:
                            rhs = es[:, c * CHUNK:c * CHUNK + cw]
                            base = base_s
                        else:
                            o2 = (c - ns_) * CHUNK
                            rhs = ev[:, o2:o2 + cw]
                            base = base_v
                        gi = g // 32
                        pt, rb = ps[(dist, gi)]
                        nc.tensor.matmul(pt[rb:rb + 32, 0:cw],
                                         w_t[:, base - r:base - r + 32],
                                         rhs, start=g % 32 == 0,
                                         stop=g == min(32 * gi + 31,
                                                       NCHUNK - 1),
                                         skip_group_check=True)
                # drain any group whose chunks were all emitted one tile ago
                # (one tile late so the copies' deps are long retired);
                # mid-stream drains ride the idle SWDGE queue
                # G0+G1 drain as one [64,512] copy per dist (copy cost is
                # per-column, so fusing partition ranges is free); G2 as
                # [32,512]; one tile late so the copies' deps are long
                # retired. sn on vector, sa on scalar; out-DMAs on the idle
                # SWDGE queue.
                if ti == GROUP_FLUSH_TILE[1] + 1 and 1 not in flushed:
                    flushed.add(1)
                    pt0, _ = ps[(0, 0)]
                    pt1, _ = ps[(1, 0)]
                    nc.vector.tensor_copy(sb01_n[:], pt0[0:64, :])
                    nc.scalar.copy(sb01_a[:], pt1[0:64, :])
                    nc.gpsimd.dma_start(sn_out[0:64, :], sb01_n[:])
                    nc.gpsimd.dma_start(sa_out[0:64, :], sb01_a[:])
                if ti == GROUP_FLUSH_TILE[2] + 1 and 2 not in flushed:
                    flushed.add(2)
                    b, e = GROUPS[2]
                    pt0, rb0 = ps[(0, 2)]
                    pt1, rb1 = ps[(1, 2)]
                    nc.vector.tensor_copy(sb[(0, 2)][:], pt0[rb0:rb0 + 32, :])
                    nc.scalar.copy(sb[(1, 2)][:], pt1[rb1:rb1 + 32, :])
                    nc.gpsimd.dma_start(sn_out[b:e, :], sb[(0, 2)][:])
                    nc.gpsimd.dma_start(sa_out[b:e, :], sb[(1, 2)][:])

            # final small drain: [27,512] per dist; parallel chains
            # vector->sync-HWDGE and scalar->SWDGE
            gi = NGRP - 1
            b, e = GROUPS[gi]
            pt0, rb0 = ps[(0, gi)]
            pt1, rb1 = ps[(1, gi)]
            nc.vector.tensor_copy(sb[(0, gi)][:], pt0[rb0:rb0 + (e - b), :])
            nc.scalar.copy(sb[(1, gi)][:], pt1[rb1:rb1 + (e - b), :])
            nc.sync.dma_start(sn_out[b:e, :], sb[(0, gi)][:])
            nc.gpsimd.dma_start(sa_out[b:e, :], sb[(1, gi)][:])

    nc.compile()
    return nc


def _get_compiled():
    global _COMPILED
    if _COMPILED is None:
        _COMPILED = _build()
    return _COMPILED


def kernel(encoded, normal_dist, anomaly_dist):
    global LAST_RESULTS
    import ml_dtypes
    from concourse.bass_utils import run_bass_kernel_spmd

    x = np.ascontiguousarray(np.asarray(encoded, dtype=np.float32))
    nd = np.asarray(normal_dist, dtype=np.float64)
    ad = np.asarray(anomaly_dist, dtype=np.float64)

    # per-dim stats (torch defaults: unbiased std)
    mu_n = nd.mean(axis=1)
    sd_n = nd.std(axis=1, ddof=1)
    mu_a = ad.mean(axis=1)
    sd_a = ad.std(axis=1, ddof=1)
    isd_n, isd_a = 1.0 / sd_n, 1.0 / sd_a

    inv_sqrt2 = 1.0 / np.sqrt(2.0)
    scale_n = isd_n * inv_sqrt2
    bias_n = -mu_n * isd_n * inv_sqrt2
    scale_a = isd_a * inv_sqrt2
    bias_a = -mu_a * isd_a * inv_sqrt2
    consts = np.stack([
        scale_n, bias_n, scale_a, bias_a,
        K_DVE * scale_n, K_DVE * bias_n,
        K_DVE * scale_a, K_DVE * bias_a,
    ], axis=1).astype(np.float32)     # [128, 8]

    half_sqrt_pi = 0.5 * np.sqrt(np.pi)
    wmat = np.zeros((DIM, W_COLS), dtype=ml_dtypes.bfloat16)
    wmat[:, ACT_N_BASE] = (INV_SQRT_2PI * isd_n * half_sqrt_pi).astype(
        ml_dtypes.bfloat16)
    wmat[:, ACT_A_BASE] = (INV_SQRT_2PI * isd_a * half_sqrt_pi).astype(
        ml_dtypes.bfloat16)
    wmat[:, DVE_N_BASE] = (INV_SQRT_2PI * isd_n).astype(ml_dtypes.bfloat16)
    wmat[:, DVE_A_BASE] = (INV_SQRT_2PI * isd_a).astype(ml_dtypes.bfloat16)

    in_maps = []
    for i in range(NCORES):
        lo = i * R
        shard_T = np.ascontiguousarray(x[lo:lo + R].T)   # [128, R]
        in_maps.append({"xT": shard_T, "consts": consts, "wmat": wmat})

    nc = _get_compiled()
    try:
        res = run_bass_kernel_spmd(nc, in_maps, core_ids=list(range(NCORES)))
    except Exception:
        # one retry: the NRT occasionally reports a transient
        # NRT_EXEC_UNIT_UNRECOVERABLE on an otherwise-healthy device
        res = run_bass_kernel_spmd(nc, in_maps, core_ids=list(range(NCORES)))
    LAST_RESULTS = res

    s_n = np.empty(N, dtype=np.float64)
    s_a = np.empty(N, dtype=np.float64)
    for i in range(NCORES):
        lo = i * R
        s_n[lo:lo + R] = res.results[i]["sn_out"].reshape(-1)[:R]
        s_a[lo:lo + R] = res.results[i]["sa_out"].reshape(-1)[:R]

    # exact recurrence p_k = (p_{k-1} + s_k)/dim as truncated causal
    # convolution: p_k = sum_j (1/dim)^(j+1) s_{k-j}; (1/128)^14 ~ 3e-30.
    a = 1.0 / DIM
    pn = np.zeros(N, dtype=np.float64)
    pa = np.zeros(N, dtype=np.float64)
    wgt = a
    for j in range(14):
        if j == 0:
            pn += wgt * s_n
            pa += wgt * s_a
        else:
            pn[j:] += wgt * s_n[:-j]
            pa[j:] += wgt * s_a[:-j]
        wgt *= a
    total = pn + pa
    out = np.empty((N, 2), dtype=np.float32)
    out[:, 0] = (pn / total).astype(np.float32)
    out[:, 1] = (pa / total).astype(np.float32)
    return out


# revision 32
# speedup vs baseline: 1.1662x; 1.1662x over previous
"""Trainium2 Bass kernel for nn_ClassificationLayer (Gaussian pdf-sum classifier).

Math:
  mu/sd per dim from tiny [128,10] reference sets (host, exact).
  Per row i: s_n[i] = sum_d INV_SQRT_2PI/sd_d * exp(-0.5*((x[i,d]-mu_d)/sd_d)^2)
  (same for anomaly), then the batch recurrence p_k = (p_{k-1} + s_k)/128,
  output = [pn/(pn+pa), pa/(pn+pa)].

Device strategy (8 cores, data-parallel over N, exact 62500-row shards):
  - Host transposes each core's shard to [128 dims, R rows]; per-dim constants
    become per-partition scale/bias vectors.
  - The elementwise Gaussian is split across TWO engines so neither is the
    bottleneck:
      * ScalarE: one ACTIVATE per distribution per tile computes
        Derivative_Erf(scale*x + bias) = (2/sqrt(pi)) * exp(-((x-mu)/sd)^2/2),
        output in bf16.  ~1.04 ns per column per distribution.
      * VectorE: a custom fused DVE op (registered at import) computes
        Schraudolph exp bits: out_i16 = max(BETA - (a'x + b')^2, 0) converted
        to int16, which *is* the bf16 bit pattern of ~exp(-((x-mu)/sd)^2/2).
        ~1.26 ns per column per distribution.  Max per-element error ~3%
        (sawtooth); uniform bias cancels in the output ratio.
  - Reduction over dims (partitions) via TensorEngine matvec in bf16.  The
    stationary operand is a 32-wide shifted window over a zero-padded bf16
    weight buffer so chunk g's sums land in PSUM row g%32 of its 32-chunk
    accumulation group (matmul output base partitions are restricted to
    {0,32,64}, so groups are 32 wide).  Groups [0,32), [32,64), [64,96)
    drain under compute on the SWDGE queue as each completes; only a
    [27,512] group per dist remains at the end.
  - The x stream owns the Sync HWDGE queue exclusively (descriptors in pure
    stream order); consts ride the Scalar HWDGE queue; weights are [128,4]
    bf16 (512B).  Final two 14KB drains go on sync+scalar HWDGE in parallel.
  - The scalar recurrence decays by 1/128 per step, so it is re-run exactly on
    the gathered per-row sums on host as a short causal convolution in float64.
"""

import numpy as np

N, DIM, S = 500000, 128, 10
INV_SQRT_2PI = 0.3989422804014327
NCORES = 8
CHUNK = 512                      # rows per matvec (PSUM bank free-dim)
R = N // NCORES                  # 62500 rows per core, exact (no padding)
NCHUNK = -(-R // CHUNK)          # 123 chunks; last chunk has 36 rows
LAST_W = R - (NCHUNK - 1) * CHUNK

# Tile plan: (n_chunks, n_dve_chunks). ScalarE takes the FIRST nc-nd chunks
# of each tile, DVE the rest; each half is DMA'd separately so each engine
# only waits for its own half (halves the pipeline granularity). Engine
# per-column cost degrades beyond ~2560 cols/instruction, so 8-chunk tiles
# (2048-col instructions) are the sweet spot; small head tiles start both
# engines early, small tail tiles keep the post-stream compute lag tiny.
# DVE share 56/123 balances DVE (~1.09 marginal ns/col) vs ScalarE (~0.88).
TILE_PLAN = ([(2, 1), (4, 2)]
             + [(8, 4), (8, 3), (8, 4), (8, 4), (8, 3), (8, 4), (8, 4),
                (8, 3), (8, 4), (8, 4), (8, 3), (8, 4), (8, 3)]
             + [(6, 3), (4, 2), (2, 1), (1, 0)])
assert sum(t[0] for t in TILE_PLAN) == NCHUNK
assert sum(t[1] for t in TILE_PLAN) == 56
TILES = []           # (chunk0, n_chunks, n_dve)
_g = 0
for _nc_, _nd in TILE_PLAN:
    TILES.append((_g, _nc_, _nd))
    _g += _nc_
MAX_WV = max(nd * CHUNK for _, nd in TILE_PLAN)
MAX_WS = max((nc_ - nd) * CHUNK for nc_, nd in TILE_PLAN)
MAX_W = max(nc_ * CHUNK for nc_, _ in TILE_PLAN)

# PSUM accumulation groups: 32 chunks each (matmul base partitions must be
# in {0,32,64}); chunk g lands in row g%32 of group g//32. Groups 0-2 drain
# mid-stream on the SWDGE queue; group 3 ([27,512]) is the small final drain.
GROUPS = [(0, 32), (32, 64), (64, 96), (96, NCHUNK)]     # [base, end)
NGRP = len(GROUPS)
# tile index after which each group's chunks are all emitted
_cum = 0
GROUP_FLUSH_TILE = {}
for _ti, (_, _nc_, _) in enumerate(TILES):
    _cum += _nc_
    for _gi, (_b, _e) in enumerate(GROUPS):
        if _cum >= _e and _gi not in GROUP_FLUSH_TILE:
            GROUP_FLUSH_TILE[_gi] = _ti

# weight-window bases in the [128, 256] weight buffer (c vector at the base
# column, zeros elsewhere; window [base-r, base-r+32) puts c at PSUM row r;
# adjacent windows span [base-31, base+32) and never reach another base)
ACT_N_BASE, ACT_A_BASE, DVE_N_BASE, DVE_A_BASE = 32, 96, 160, 224
W_COLS = 256

# Schraudolph-in-bf16-bit-space constants.
LOG2E = 1.4426950408889634
K_DVE = float(np.sqrt(128.0 * LOG2E))       # folds 2^7*log2(e) into a', b'
C_CENTER = 1.5 - 1.0 / float(np.log(2.0))   # zero-mean sawtooth offset
BETA = float(128.0 * (127.0 - C_CENTER))

_COMPILED = None
_DVE_OP = None
LAST_RESULTS = None  # BassKernelResults of the most recent device run


def _register_dve_op():
    """Register the fused Schraudolph-exp custom DVE op at runtime."""
    global _DVE_OP
    if _DVE_OP is not None:
        return _DVE_OP
    import concourse.dve_ops as dvo
    from concourse.dve_spec import Spec, Src0, C0, C1, C2, Zero, maxx, sq, lower
    from concourse.dve_spec import _has_src1
    from concourse.dve_uop import DveOpSpec

    name = "GAUSS_EXP_BITS_ANT"
    if name in dvo._SUB_OPCODE_FOR_NAME:
        _DVE_OP = next(op for op in dvo.OPS if op.name == name)
        return _DVE_OP
    t = Src0 * C0 + C1
    spec = Spec(
        body=maxx(C2 - sq(t), Zero),
        reference=lambda in0, in1, s0, s1, imm2: np.maximum(
            np.float32(imm2)
            - (in0 * s0 + s1).astype(np.float32) ** 2,
            np.float32(0.0),
        ),
    )
    row = dvo._CUSTOM_DVE_ROW_BASE + len(dvo.OPS)
    shas = {}
    for ver in ("v3", "v4"):
        try:
            uops = lower(spec, ver=ver)
            shas[ver] = DveOpSpec(
                name=name, opcode=row, uops=uops, rd1_en=_has_src1(spec)
            ).sha(ver)
        except Exception:
            pass
    op = dvo.DveOp(name, spec, subdim=False, uops_sha=shas)
    dvo.OPS.append(op)
    dvo._SUB_OPCODE_FOR_NAME[name] = row
    dvo.CUSTOM_DVE_SPECS[name] = spec
    _DVE_OP = op
    return op


def _build():
    import concourse.tile as tile
    from concourse import bacc, mybir

    dve_op = _register_dve_op()

    nc = bacc.Bacc("TRN2", target_bir_lowering=False, debug=False,
                   num_devices=NCORES)

    xT = nc.dram_tensor("xT", [DIM, R], mybir.dt.float32,
                        kind="ExternalInput").ap()
    # consts cols: 0 scale_n, 1 bias_n, 2 scale_a, 3 bias_a (ACT);
    #              4 a'_n, 5 b'_n, 6 a'_a, 7 b'_a (DVE, scaled by K_DVE)
    consts = nc.dram_tensor("consts", [DIM, 8], mybir.dt.float32,
                            kind="ExternalInput").ap()
    wmat = nc.dram_tensor("wmat", [DIM, W_COLS], mybir.dt.bfloat16,
                          kind="ExternalInput").ap()
    sn_out = nc.dram_tensor("sn_out", [128, CHUNK], mybir.dt.float32,
                            kind="ExternalOutput").ap()
    sa_out = nc.dram_tensor("sa_out", [128, CHUNK], mybir.dt.float32,
                            kind="ExternalOutput").ap()

    DErf = mybir.ActivationFunctionType.Derivative_Erf
    bf16 = mybir.dt.bfloat16

    with tile.TileContext(nc) as tc:
        with tc.tile_pool(name="cpool", bufs=1) as cpool, \
             tc.tile_pool(name="xpool", bufs=6) as xpool, \
             tc.tile_pool(name="evpool", bufs=3) as evpool, \
             tc.tile_pool(name="espool", bufs=3) as espool, \
             tc.tile_pool(name="pspool", bufs=1, space="PSUM") as pspool:

            # x tiles first on the Sync HWDGE queue: the stream's first
            # descriptor is x tile 0 (consts ride the Scalar HWDGE queue).
            # Each tile is fetched as two DMAs — ScalarE's half (cols
            # [0,ws)) then DVE's half — so ACT only waits for its own half,
            # which lands ~half a tile earlier in the stream.
            def fetch_x(ti, name=None):
                off, nch, ndv = TILES[ti]
                off *= CHUNK
                w = min(nch * CHUNK, R - off)
                ws = w - ndv * CHUNK
                x_t = xpool.tile([DIM, w], mybir.dt.float32, tag="x",
                                 padded_shape=[DIM, MAX_W],
                                 name=name or f"x_{ti}")
                if ws >= 2 * CHUNK and w - ws >= 2 * CHUNK:
                    nc.sync.dma_start(x_t[:, 0:ws], xT[:, off:off + ws])
                    nc.sync.dma_start(x_t[:, ws:w], xT[:, off + ws:off + w])
                else:
                    nc.sync.dma_start(x_t[:], xT[:, off:off + w])
                return x_t

            x_pre = {ti: fetch_x(ti, name=f"x_pre{ti}") for ti in (0, 1)}
            consts_t = cpool.tile([DIM, 8], mybir.dt.float32)
            nc.scalar.dma_start(consts_t[:], consts[:, :])
            # Dummy activation: triggers the erf_derivative table load while
            # the first x tiles are still in flight. memset instead of a
            # consts read so the table load has no DMA dependency at all.
            warm_t = cpool.tile([DIM, 1], mybir.dt.float32)
            nc.vector.memset(warm_t[:], 0.0)
            nc.scalar.activation(warm_t[:], warm_t[:], DErf,
                                 bias=0.0, scale=1.0)
            # weights via SWDGE, off both HWDGE queues
            w_t = cpool.tile([DIM, W_COLS], bf16)
            nc.gpsimd.dma_start(w_t[:], wmat[:, :])

            # per dist: two [64,512] PSUM tiles, each holding two 32-chunk
            # accumulation groups at base partitions 0 and 32 (both legal
            # matmul output bases); chunk g accumulates into row g%32 of
            # group g//32.
            ps = {}   # (dist, group) -> (tile, row_base)
            sb = {}   # (dist, group) -> staging tile
            for d in (0, 1):
                for half in (0, 1):
                    pt = pspool.tile([64, CHUNK], mybir.dt.float32,
                                     name=f"ps_{d}_{half}")
                    ps[(d, 2 * half)] = (pt, 0)
                    ps[(d, 2 * half + 1)] = (pt, 32)
                for gi in (2, 3):
                    b, e = GROUPS[gi]
                    sb[(d, gi)] = cpool.tile([e - b, CHUNK],
                                             mybir.dt.float32,
                                             name=f"sb_{d}_{gi}")
            sb01_n = cpool.tile([64, CHUNK], mybir.dt.float32)
            sb01_a = cpool.tile([64, CHUNK], mybir.dt.float32)

            # PE emission is ascending global chunk order per dist (ACT
            # chunks come first within each tile), so group gi's first/last
            # emitted chunks are simply its boundary chunks.
            flushed = set()
            for ti, (g0, nch, ndv) in enumerate(TILES):
                off = g0 * CHUNK
                w = min(nch * CHUNK, R - off)
                ns_ = nch - ndv
                ws = min(ns_ * CHUNK, w)
                wv = w - ws
                x_t = x_pre[ti] if ti in x_pre else fetch_x(ti)
                # Two waves per tile: produce n-dist results, run n matmuls
                # while the a-dist results are being produced, then a
                # matmuls. ACT chunks first within each wave. The last two
                # tiles process dist a FIRST so its slower final drain chain
                # (scalar copy -> SWDGE) starts earlier and both output
                # chains finish together.
                d_order = (1, 0) if ti >= len(TILES) - 2 else (0, 1)
                for dist in d_order:
                    ev = es = None
                    if ws:
                        es = espool.tile([DIM, ws], bf16,
                                         tag="es" + "na"[dist],
                                         padded_shape=[DIM, MAX_WS])
                        nc.scalar.activation(es[:], x_t[:, 0:ws], DErf,
                                             bias=consts_t[:, 1 + 2 * dist:
                                                           2 + 2 * dist],
                                             scale=consts_t[:, 2 * dist:
                                                            1 + 2 * dist])
                    if wv:
                        ev = evpool.tile([DIM, wv], bf16,
                                         tag="ev" + "na"[dist],
                                         padded_shape=[DIM, MAX_WV])
                        nc.vector._custom_dve(
                            dve_op, out=ev[:].bitcast(mybir.dt.int16),
                            in0=x_t[:, ws:w],
                            s0=consts_t[:, 4 + 2 * dist:5 + 2 * dist],
                            s1=consts_t[:, 5 + 2 * dist:6 + 2 * dist],
                            imm2=BETA)
                    base_v = DVE_N_BASE if dist == 0 else DVE_A_BASE
                    base_s = ACT_N_BASE if dist == 0 else ACT_A_BASE
                    for c in range(nch):
                        g = g0 + c
                        r = g % 32
                        cw = min(CHUNK, w - c * CHUNK)
                        if c < ns_:
                            rhs = es[:, c * CHUNK:c * CHUNK + cw]
                            base = base_s
                        else:
                            o2 = (c - ns_) * CHUNK
                            rhs = ev[:, o2:o2 + cw]
                            base = base_v
                        gi = g // 32
                        pt, rb = ps[(dist, gi)]
                        nc.tensor.matmul(pt[rb:rb + 32, 0:cw],
                                         w_t[:, base - r:base - r + 32],
                                         rhs, start=g % 32 == 0,
                                         stop=g == min(32 * gi + 31,
                                                       NCHUNK - 1),
                                         skip_group_check=True)
                # drain any group whose chunks were all emitted one tile ago
                # (one tile late so the copies' deps are long retired);
                # mid-stream drains ride the idle SWDGE queue
                # G0+G1 drain as one [64,512] copy per dist (copy cost is
                # per-column, so fusing partition ranges is free); G2 as
                # [32,512]; one tile late so the copies' deps are long
                # retired. sn on vector, sa on scalar; out-DMAs on the idle
                # SWDGE queue.
                if ti == GROUP_FLUSH_TILE[1] + 1 and 1 not in flushed:
                    flushed.add(1)
                    pt0, _ = ps[(0, 0)]
                    pt1, _ = ps[(1, 0)]
                    nc.vector.tensor_copy(sb01_n[:], pt0[0:64, :])
                    nc.scalar.copy(sb01_a[:], pt1[0:64, :])
                    nc.gpsimd.dma_start(sn_out[0:64, :], sb01_n[:])
                    nc.gpsimd.dma_start(sa_out[0:64, :], sb01_a[:])
                if ti == GROUP_FLUSH_TILE[2] + 1 and 2 not in flushed:
                    flushed.add(2)
                    b, e = GROUPS[2]
                    pt0, rb0 = ps[(0, 2)]
                    pt1, rb1 = ps[(1, 2)]
                    nc.vector.tensor_copy(sb[(0, 2)][:], pt0[rb0:rb0 + 32, :])
                    nc.scalar.copy(sb[(1, 2)][:], pt1[rb1:rb1 + 32, :])
                    nc.gpsimd.dma_start(sn_out[b:e, :], sb[(0, 2)][:])
                    nc.gpsimd.dma_start(sa_out[b:e, :], sb[(1, 2)][:])

            # final small drain: [27,512] per dist in column halves so the
            # first half's DMA pipe latency overlaps the second half's copy;
            # parallel chains vector->sync-HWDGE (sn) and scalar->SWDGE (sa)
            gi = NGRP - 1
            b, e = GROUPS[gi]
            pt0, rb0 = ps[(0, gi)]
            pt1, rb1 = ps[(1, gi)]
            h2 = CHUNK // 2
            for lo, hi in ((0, h2), (h2, CHUNK)):
                nc.scalar.copy(sb[(1, gi)][:, lo:hi],
                               pt1[rb1:rb1 + (e - b), lo:hi])
                nc.vector.tensor_copy(sb[(0, gi)][:, lo:hi],
                                      pt0[rb0:rb0 + (e - b), lo:hi])
                nc.gpsimd.dma_start(sa_out[b:e, lo:hi],
                                    sb[(1, gi)][:, lo:hi])
                nc.sync.dma_start(sn_out[b:e, lo:hi],
                                  sb[(0, gi)][:, lo:hi])

    nc.compile()
    return nc


def _get_compiled():
    global _COMPILED
    if _COMPILED is None:
        _COMPILED = _build()
    return _COMPILED


def kernel(encoded, normal_dist, anomaly_dist):
    global LAST_RESULTS
    import ml_dtypes
    from concourse.bass_utils import run_bass_kernel_spmd

    x = np.ascontiguousarray(np.asarray(encoded, dtype=np.float32))
    nd = np.asarray(normal_dist, dtype=np.float64)
    ad = np.asarray(anomaly_dist, dtype=np.float64)

    # per-dim stats (torch defaults: unbiased std)
    mu_n = nd.mean(axis=1)
    sd_n = nd.std(axis=1, ddof=1)
    mu_a = ad.mean(axis=1)
    sd_a = ad.std(axis=1, ddof=1)
    isd_n, isd_a = 1.0 / sd_n, 1.0 / sd_a

    inv_sqrt2 = 1.0 / np.sqrt(2.0)
    scale_n = isd_n * inv_sqrt2
    bias_n = -mu_n * isd_n * inv_sqrt2
    scale_a = isd_a * inv_sqrt2
    bias_a = -mu_a * isd_a * inv_sqrt2
    consts = np.stack([
        scale_n, bias_n, scale_a, bias_a,
        K_DVE * scale_n, K_DVE * bias_n,
        K_DVE * scale_a, K_DVE * bias_a,
    ], axis=1).astype(np.float32)     # [128, 8]

    half_sqrt_pi = 0.5 * np.sqrt(np.pi)
    wmat = np.zeros((DIM, W_COLS), dtype=ml_dtypes.bfloat16)
    wmat[:, ACT_N_BASE] = (INV_SQRT_2PI * isd_n * half_sqrt_pi).astype(
        ml_dtypes.bfloat16)
    wmat[:, ACT_A_BASE] = (INV_SQRT_2PI * isd_a * half_sqrt_pi).astype(
        ml_dtypes.bfloat16)
    wmat[:, DVE_N_BASE] = (INV_SQRT_2PI * isd_n).astype(ml_dtypes.bfloat16)
    wmat[:, DVE_A_BASE] = (INV_SQRT_2PI * isd_a).astype(ml_dtypes.bfloat16)

    in_maps = []
    for i in range(NCORES):
        lo = i * R
        shard_T = np.ascontiguousarray(x[lo:lo + R].T)   # [128, R]
        in_maps.append({"xT": shard_T, "consts": consts, "wmat": wmat})

    nc = _get_compiled()
    try:
        res = run_bass_kernel_spmd(nc, in_maps, core_ids=list(range(NCORES)))
    except Exception:
        # one retry: the NRT occasionally reports a transient
        # NRT_EXEC_UNIT_UNRECOVERABLE on an otherwise-healthy device
        res = run_bass_kernel_spmd(nc, in_maps, core_ids=list(range(NCORES)))
    LAST_RESULTS = res

    s_n = np.empty(N, dtype=np.float64)
    s_a = np.empty(N, dtype=np.float64)
    for i in range(NCORES):
        lo = i * R
        s_n[lo:lo + R] = res.results[i]["sn_out"].reshape(-1)[:R]
        s_a[lo:lo + R] = res.results[i]["sa_out"].reshape(-1)[:R]

    # exact recurrence p_k = (p_{k-1} + s_k)/dim as truncated causal
    # convolution: p_k = sum_j (1/dim)^(j+1) s_{k-j}; (1/128)^14 ~ 3e-30.
    a = 1.0 / DIM
    pn = np.zeros(N, dtype=np.float64)
    pa = np.zeros(N, dtype=np.float64)
    wgt = a
    for j in range(14):
        if j == 0:
            pn += wgt * s_n
            pa += wgt * s_a
        else:
            pn[j:] += wgt * s_n[:-j]
            pa[j:] += wgt * s_a[:-j]
        wgt *= a
    total = pn + pa
    out = np.empty((N, 2), dtype=np.float32)
    out[:, 0] = (pn / total).astype(np.float32)
    out[:, 1] = (pa / total).astype(np.float32)
    return out
